# revision 4
# baseline (speedup 1.0000x reference)
"""Trainium2 Bass kernel for nn_BasicConv2d (int8 conv + global requant + BN + requant + ReLU).

Self-contained: takes full inputs, shards batch dim over 8 NeuronCores,
runs one SPMD Bass program (conv as 9 shifted matmuls, tiny AllGathers for
the global max / BN-stat reductions), gathers full output.
"""
import numpy as np
import ml_dtypes

import jax  # noqa: F401  (axon PJRT backend provides the 8 NeuronCores)

try:
    jax.config.update("jax_compilation_cache_dir", "/tmp/jaxcache")
    jax.config.update("jax_persistent_cache_min_compile_time_secs", 0.0)
except Exception:
    pass

import concourse.bass as bass
import concourse.tile as tile
from concourse import mybir, bacc
from concourse.bass_utils import run_bass_kernel_spmd

F32 = mybir.dt.float32
I32 = mybir.dt.int32
I8 = mybir.dt.int8
BF16 = mybir.dt.bfloat16
AF = mybir.ActivationFunctionType
OP = mybir.AluOpType
AX = mybir.AxisListType

N, CIN, H, W = 32, 128, 56, 56
COUT, KH, KW = 256, 3, 3
OH, OW = 54, 54
PX = OH * OW            # 2916
NCORES = 8
NIMG = N // NCORES      # 4 images per core
NRB = 6                 # row blocks per image (9 output rows each)
RBPX = PX // NRB        # 486 = 9 rows * 54 cols
HALFS = 2               # two 128-channel halves of COUT
COLS_H = NIMG * PX      # 11664 columns per half
COLS = HALFS * COLS_H   # 23328
EPS = 1e-5
RG = [list(range(NCORES))]

_cached = {}


def _col(h, i, rb=0):
    return (h * NIMG + i) * PX + rb * RBPX


def _bitexp_pow2(nc, pool, r_ap, name):
    """Given r [1,1] f32 (>0), return (s [1,1] f32 = 2^(7-ceil(log2 r)),
    bwb [1,1] i32 = ceil(log2 r) + 127). Exact integer/bit arithmetic."""
    ri = r_ap.bitcast(I32)
    eb = pool.tile([1, 1], I32, tag=f"{name}_eb")
    nc.vector.tensor_scalar(eb[:], ri, 23, 0xFF, OP.logical_shift_right, OP.bitwise_and)
    mant = pool.tile([1, 1], I32, tag=f"{name}_mant")
    nc.vector.tensor_scalar(mant[:], ri, 0x7FFFFF, None, OP.bitwise_and)
    nz = pool.tile([1, 1], I32, tag=f"{name}_nz")
    nc.vector.tensor_scalar(nz[:], mant[:], 0, None, OP.is_gt)
    bwb = pool.tile([1, 1], I32, tag=f"{name}_bwb")
    nc.vector.tensor_tensor(bwb[:], eb[:], nz[:], OP.add)
    t = pool.tile([1, 1], I32, tag=f"{name}_t")
    nc.vector.tensor_scalar(t[:], bwb[:], -1, 261, OP.mult, OP.add)  # 261 - bwb
    sb = pool.tile([1, 1], I32, tag=f"{name}_sb")
    nc.vector.tensor_scalar(sb[:], t[:], 23, None, OP.logical_shift_left)
    s = pool.tile([1, 1], F32, tag=f"{name}_s")
    nc.vector.tensor_copy(s[:], sb[:].bitcast(F32))
    return s, bwb


def _pow2_from_int(nc, pool, oi_ap, name):
    """2^k for k given as [1,1] int32 (normal range)."""
    b = pool.tile([1, 1], I32, tag=f"{name}_b")
    nc.vector.tensor_scalar(b[:], oi_ap, 127, None, OP.add)
    bs = pool.tile([1, 1], I32, tag=f"{name}_bs")
    nc.vector.tensor_scalar(bs[:], b[:], 23, None, OP.logical_shift_left)
    p = pool.tile([1, 1], F32, tag=f"{name}_p")
    nc.vector.tensor_copy(p[:], bs[:].bitcast(F32))
    return p


def _broadcast(nc, dram_pool, sbuf_pool, src_ap, name):
    """[1,1] f32 SBUF value -> [128,1] f32 SBUF (all partitions)."""
    d = dram_pool.tile([1], F32, tag=f"{name}_d")
    nc.sync.dma_start(d[:], src_ap)
    b = sbuf_pool.tile([128, 1], F32, tag=f"{name}_b")
    bcast = bass.AP(tensor=d.tensor, offset=d[:].offset, ap=[[0, 128], [1, 1]])
    nc.sync.dma_start(b[:], bcast)
    return b


def _build():
    nc = bacc.Bacc("TRN2", target_bir_lowering=False, debug=False, num_devices=NCORES)

    x_in = nc.dram_tensor("x", [NIMG, CIN, H * W], BF16, kind="ExternalInput")
    w_in = nc.dram_tensor("w", [KH * KW, CIN, COUT], BF16, kind="ExternalInput")
    scal_in = nc.dram_tensor("scal", [1, 1], F32, kind="ExternalInput")  # x_exp+w_exp
    gamma_in = nc.dram_tensor("gamma2", [HALFS, 128], F32, kind="ExternalInput")
    beta_in = nc.dram_tensor("beta2", [HALFS, 128], F32, kind="ExternalInput")
    out_val = nc.dram_tensor("out_val", [NIMG, COUT, PX], I8, kind="ExternalOutput")
    out_exp = nc.dram_tensor("out_exp", [1, 1], F32, kind="ExternalOutput")

    with tile.TileContext(nc) as tc:
        with (
            tc.tile_pool(name="big", bufs=1) as big,
            tc.tile_pool(name="stat", bufs=1) as stat,
            tc.tile_pool(name="sc", bufs=1) as sc,
            tc.tile_pool(name="dram", bufs=1, space="DRAM") as dram,
            tc.tile_pool(name="psum", bufs=8, space="PSUM") as psum_pool,
        ):
            # ---- load inputs to SBUF ----
            x_sb = big.tile([128, NIMG, H * W], BF16)
            nc.sync.dma_start(x_sb[:], x_in[:].rearrange("i p c -> p i c"))
            w_sb = big.tile([128, KH * KW, COUT], BF16)
            nc.sync.dma_start(w_sb[:], w_in[:].rearrange("k p c -> p k c"))
            gam_sb = stat.tile([128, HALFS], F32)
            nc.sync.dma_start(gam_sb[:], gamma_in[:].rearrange("h p -> p h"))
            bet_sb = stat.tile([128, HALFS], F32)
            nc.sync.dma_start(bet_sb[:], beta_in[:].rearrange("h p -> p h"))
            scal_sb = sc.tile([1, 1], F32)
            nc.sync.dma_start(scal_sb[:], scal_in[:])

            acc_sb = big.tile([128, COLS], F32)
            q_sb = big.tile([128, COLS], I8)
            o_sb = big.tile([128, COLS], I8)

            mx_raw = stat.tile([128, HALFS, NIMG * NRB], F32)
            mn_raw = stat.tile([128, HALFS, NIMG * NRB], F32)

            # ---- phase 1: conv (9 shifted matmuls per psum tile) ----
            for i in range(NIMG):
                x_img = x_sb[:, i, :].rearrange("p (r c) -> p r c", c=W)
                for rb in range(NRB):
                    for h in range(HALFS):
                        ps = psum_pool.tile([128, RBPX], F32)
                        for k in range(KH * KW):
                            kh, kw = divmod(k, KW)
                            rhs = x_img[:, rb * 9 + kh : rb * 9 + kh + 9, kw : kw + OW]
                            nc.tensor.matmul(
                                ps[:],
                                w_sb[:, k, h * 128 : (h + 1) * 128],
                                rhs,
                                start=(k == 0),
                                stop=(k == KH * KW - 1),
                            )
                        c0 = _col(h, i, rb)
                        nc.scalar.activation(acc_sb[:, c0 : c0 + RBPX], ps[:], AF.Copy)
                        j = i * NRB + rb
                        nc.vector.tensor_reduce(
                            mx_raw[:, h, j : j + 1], ps[:], AX.X, OP.max
                        )
                        nc.vector.tensor_reduce(
                            mn_raw[:, h, j : j + 1], ps[:], AX.X, OP.min
                        )

            # per-core per-channel acc max/min
            chmax = stat.tile([128, HALFS], F32)
            nc.vector.tensor_reduce(chmax[:], mx_raw[:], AX.X, OP.max)
            chmin = stat.tile([128, HALFS], F32)
            nc.vector.tensor_reduce(chmin[:], mn_raw[:], AX.X, OP.min)

            # ---- collective 1: AllGather per-channel acc max/min ----
            cc1_in = dram.tile([2, HALFS, 128], F32)
            for h in range(HALFS):
                nc.sync.dma_start(cc1_in[0, h, :], chmax[:, h : h + 1])
                nc.sync.dma_start(cc1_in[1, h, :], chmin[:, h : h + 1])
            cc1_out = dram.tile([NCORES, 2, HALFS, 128], F32)
            nc.gpsimd.collective_compute(
                "AllGather", OP.bypass, replica_groups=RG,
                ins=[cc1_in[:].opt()], outs=[cc1_out[:].opt()],
            )

            # r1 = absmax over everything gathered
            g1 = sc.tile([1, NCORES * 2 * HALFS * 128], F32)
            nc.sync.dma_start(g1[:], cc1_out[:].rearrange("a b c d -> (a b c d)")[None, :])
            r1 = sc.tile([1, 1], F32)
            nc.vector.tensor_reduce(r1[:], g1[:], AX.X, OP.max, apply_absolute_value=True)
            r1m = sc.tile([1, 1], F32)
            nc.vector.tensor_scalar(r1m[:], r1[:], 1e-30, None, OP.max)
            s1, bwb1 = _bitexp_pow2(nc, sc, r1m[:], "s1")
            s1_b = _broadcast(nc, dram, stat, s1[:], "s1")

            # oe = (x_exp+w_exp) + bw1 - 7 ;  poe = 2^oe, poe2 = 2^(2*oe)
            sxw_i = sc.tile([1, 1], I32)
            nc.vector.tensor_copy(sxw_i[:], scal_sb[:])
            oe_i = sc.tile([1, 1], I32)
            nc.vector.tensor_scalar(oe_i[:], bwb1[:], 1, -134, OP.mult, OP.add)
            nc.vector.tensor_tensor(oe_i[:], oe_i[:], sxw_i[:], OP.add)
            oe2_i = sc.tile([1, 1], I32)
            nc.vector.tensor_scalar(oe2_i[:], oe_i[:], 2, None, OP.mult)
            poe = _pow2_from_int(nc, sc, oe_i[:], "poe")
            poe2 = _pow2_from_int(nc, sc, oe2_i[:], "poe2")
            poe_b = _broadcast(nc, dram, stat, poe[:], "poe")
            poe2_b = _broadcast(nc, dram, stat, poe2[:], "poe2")

            # global per-channel acc extremes -> q extremes
            gmax = stat.tile([128, HALFS, NCORES], F32)
            gmin = stat.tile([128, HALFS, NCORES], F32)
            for h in range(HALFS):
                src_mx = bass.AP(
                    tensor=cc1_out.tensor, offset=cc1_out[:].offset + h * 128,
                    ap=[[1, 128], [2 * HALFS * 128, NCORES]],
                )
                nc.sync.dma_start(gmax[:, h, :], src_mx)
                src_mn = bass.AP(
                    tensor=cc1_out.tensor,
                    offset=cc1_out[:].offset + HALFS * 128 + h * 128,
                    ap=[[1, 128], [2 * HALFS * 128, NCORES]],
                )
                nc.sync.dma_start(gmin[:, h, :], src_mn)
            gchmax = stat.tile([128, HALFS], F32)
            nc.vector.tensor_reduce(gchmax[:], gmax[:], AX.X, OP.max)
            gchmin = stat.tile([128, HALFS], F32)
            nc.vector.tensor_reduce(gchmin[:], gmin[:], AX.X, OP.min)
            qmx8 = stat.tile([128, HALFS], I8)
            nc.scalar.activation(qmx8[:], gchmax[:], AF.Copy, scale=s1_b[:, 0:1])
            qmn8 = stat.tile([128, HALFS], I8)
            nc.scalar.activation(qmn8[:], gchmin[:], AF.Copy, scale=s1_b[:, 0:1])
            qmaxf = stat.tile([128, HALFS], F32)
            nc.vector.tensor_copy(qmaxf[:], qmx8[:])
            qminf = stat.tile([128, HALFS], F32)
            nc.vector.tensor_copy(qminf[:], qmn8[:])

            # ---- phase 2: q = int8(acc * s1) ; bn stats of q ----
            stats6 = stat.tile([128, HALFS, NIMG * NRB, 6], F32)
            for h in range(HALFS):
                for i in range(NIMG):
                    c0 = _col(h, i)
                    nc.scalar.activation(
                        q_sb[:, c0 : c0 + PX], acc_sb[:, c0 : c0 + PX],
                        AF.Copy, scale=s1_b[:, 0:1],
                    )
                    for rb in range(NRB):
                        cb = c0 + rb * RBPX
                        nc.vector.bn_stats(
                            stats6[:, h, i * NRB + rb, :], q_sb[:, cb : cb + RBPX]
                        )
            mv = stat.tile([128, HALFS, 2], F32)
            for h in range(HALFS):
                nc.vector.bn_aggr(mv[:, h, :], stats6[:, h, :, :])

            # ---- collective 2: AllGather per-channel (mean, var) ----
            cc2_in = dram.tile([2, HALFS, 128], F32)
            for h in range(HALFS):
                nc.sync.dma_start(cc2_in[0, h, :], mv[:, h, 0:1])
                nc.sync.dma_start(cc2_in[1, h, :], mv[:, h, 1:2])
            cc2_out = dram.tile([NCORES, 2, HALFS, 128], F32)
            nc.gpsimd.collective_compute(
                "AllGather", OP.bypass, replica_groups=RG,
                ins=[cc2_in[:].opt()], outs=[cc2_out[:].opt()],
            )
            gmean = stat.tile([128, HALFS, NCORES], F32)
            gvar = stat.tile([128, HALFS, NCORES], F32)
            for h in range(HALFS):
                src_m = bass.AP(
                    tensor=cc2_out.tensor, offset=cc2_out[:].offset + h * 128,
                    ap=[[1, 128], [2 * HALFS * 128, NCORES]],
                )
                nc.sync.dma_start(gmean[:, h, :], src_m)
                src_v = bass.AP(
                    tensor=cc2_out.tensor,
                    offset=cc2_out[:].offset + HALFS * 128 + h * 128,
                    ap=[[1, 128], [2 * HALFS * 128, NCORES]],
                )
                nc.sync.dma_start(gvar[:, h, :], src_v)

            # combine: mean_g = avg(mean_i); var_g = avg(var_i + mean_i^2) - mean_g^2
            mean_g = stat.tile([128, HALFS], F32)
            nc.vector.tensor_reduce(mean_g[:], gmean[:], AX.X, OP.add)
            nc.vector.tensor_scalar(mean_g[:], mean_g[:], 1.0 / NCORES, None, OP.mult)
            m2t = stat.tile([128, HALFS, NCORES], F32)
            nc.vector.tensor_tensor(m2t[:], gmean[:], gmean[:], OP.mult)
            nc.vector.tensor_tensor(m2t[:], m2t[:], gvar[:], OP.add)
            ex2 = stat.tile([128, HALFS], F32)
            nc.vector.tensor_reduce(ex2[:], m2t[:], AX.X, OP.add)
            nc.vector.tensor_scalar(ex2[:], ex2[:], 1.0 / NCORES, None, OP.mult)
            var_g = stat.tile([128, HALFS], F32)
            nc.vector.tensor_tensor(var_g[:], mean_g[:], mean_g[:], OP.mult)
            nc.vector.tensor_tensor(var_g[:], ex2[:], var_g[:], OP.subtract)

            # rs = rsqrt(var_g * 2^(2oe) + eps), Newton-refined
            v = stat.tile([128, HALFS], F32)
            nc.scalar.activation(v[:], var_g[:], AF.Copy, scale=poe2_b[:, 0:1])
            veps = stat.tile([128, HALFS], F32)
            nc.vector.tensor_scalar(veps[:], v[:], EPS, None, OP.add)
            eps_t = stat.tile([128, 1], F32)
            nc.vector.memset(eps_t[:], EPS)
            s_sq = stat.tile([128, HALFS], F32)
            nc.scalar.activation(s_sq[:], v[:], AF.Sqrt, bias=eps_t[:, 0:1])
            for it in range(2):
                rcp = stat.tile([128, HALFS], F32, tag=f"rcp{it}")
                nc.vector.reciprocal(rcp[:], s_sq[:])
                tn = stat.tile([128, HALFS], F32, tag=f"tn{it}")
                nc.vector.tensor_tensor(tn[:], veps[:], rcp[:], OP.mult)
                nc.vector.tensor_tensor(tn[:], tn[:], s_sq[:], OP.add)
                nc.vector.tensor_scalar(s_sq[:], tn[:], 0.5, None, OP.mult)
            rs = stat.tile([128, HALFS], F32)
            nc.vector.reciprocal(rs[:], s_sq[:])

            # A0 = 2^oe * rs * gamma ; B0 = beta - mean_g*2^oe * rs*gamma
            rg_t = stat.tile([128, HALFS], F32)
            nc.vector.tensor_tensor(rg_t[:], rs[:], gam_sb[:], OP.mult)
            a0 = stat.tile([128, HALFS], F32)
            nc.scalar.activation(a0[:], rg_t[:], AF.Copy, scale=poe_b[:, 0:1])
            mq = stat.tile([128, HALFS], F32)
            nc.scalar.activation(mq[:], mean_g[:], AF.Copy, scale=poe_b[:, 0:1])
            u = stat.tile([128, HALFS], F32)
            nc.vector.tensor_tensor(u[:], mq[:], rg_t[:], OP.mult)
            b0 = stat.tile([128, HALFS], F32)
            nc.vector.tensor_tensor(b0[:], bet_sb[:], u[:], OP.subtract)

            # r2 = max_c max(|A0*qmax+B0|, |A0*qmin+B0|)
            c1 = stat.tile([128, HALFS], F32)
            nc.vector.tensor_tensor(c1[:], a0[:], qmaxf[:], OP.mult)
            nc.vector.tensor_tensor(c1[:], c1[:], b0[:], OP.add)
            nc.scalar.activation(c1[:], c1[:], AF.Abs)
            c2 = stat.tile([128, HALFS], F32)
            nc.vector.tensor_tensor(c2[:], a0[:], qminf[:], OP.mult)
            nc.vector.tensor_tensor(c2[:], c2[:], b0[:], OP.add)
            nc.scalar.activation(c2[:], c2[:], AF.Abs)
            chr2 = stat.tile([128, HALFS], F32)
            nc.vector.tensor_tensor(chr2[:], c1[:], c2[:], OP.max)
            r2d = dram.tile([HALFS, 128], F32)
            for h in range(HALFS):
                nc.sync.dma_start(r2d[h, :], chr2[:, h : h + 1])
            g2 = sc.tile([1, HALFS * 128], F32)
            nc.sync.dma_start(g2[:], r2d[:].rearrange("a b -> (a b)")[None, :])
            r2 = sc.tile([1, 1], F32)
            nc.vector.tensor_reduce(r2[:], g2[:], AX.X, OP.max)
            r2m = sc.tile([1, 1], F32)
            nc.vector.tensor_scalar(r2m[:], r2[:], 1e-30, None, OP.max)
            s2, bwb2 = _bitexp_pow2(nc, sc, r2m[:], "s2")
            s2_b = _broadcast(nc, dram, stat, s2[:], "s2")

            # exp2 = bw2 - 7
            e2i = sc.tile([1, 1], I32)
            nc.vector.tensor_scalar(e2i[:], bwb2[:], 1, -134, OP.mult, OP.add)
            e2f = sc.tile([1, 1], F32)
            nc.vector.tensor_copy(e2f[:], e2i[:])
            nc.sync.dma_start(out_exp[:], e2f[:])

            # A' = A0*s2, B' = B0*s2
            ap_ = stat.tile([128, HALFS], F32)
            nc.scalar.activation(ap_[:], a0[:], AF.Copy, scale=s2_b[:, 0:1])
            bp_ = stat.tile([128, HALFS], F32)
            nc.scalar.activation(bp_[:], b0[:], AF.Copy, scale=s2_b[:, 0:1])

            # ---- phase 3: out = int8(relu(A'*q + B')) ----
            for h in range(HALFS):
                c0 = _col(h, 0)
                nc.scalar.activation(
                    o_sb[:, c0 : c0 + COLS_H], q_sb[:, c0 : c0 + COLS_H],
                    AF.Relu, bias=bp_[:, h : h + 1], scale=ap_[:, h : h + 1],
                )

            # ---- output DMA ----
            for i in range(NIMG):
                for h in range(HALFS):
                    c0 = _col(h, i)
                    nc.sync.dma_start(
                        out_val[i, h * 128 : (h + 1) * 128, :],
                        o_sb[:, c0 : c0 + PX],
                    )

    nc.finalize()
    return nc


def _get_nc():
    if "nc" not in _cached:
        _cached["nc"] = _build()
    return _cached["nc"]


def kernel(x_val, x_exp, w_val, w_exp, gamma, beta, _trace=False):
    nc = _get_nc()

    bf16 = ml_dtypes.bfloat16
    x = np.asarray(x_val).reshape(N, CIN, H * W).astype(bf16)
    # weights: [COUT, CIN, KH, KW] -> [KH*KW, CIN, COUT]
    w = np.ascontiguousarray(
        np.asarray(w_val).astype(np.float32).transpose(2, 3, 1, 0).reshape(KH * KW, CIN, COUT)
    ).astype(bf16)
    sxw = np.array([[np.float32(x_exp) + np.float32(w_exp)]], dtype=np.float32)
    g2 = np.ascontiguousarray(np.asarray(gamma, np.float32).reshape(HALFS, 128))
    b2 = np.ascontiguousarray(np.asarray(beta, np.float32).reshape(HALFS, 128))

    in_maps = []
    for c in range(NCORES):
        in_maps.append({
            "x": np.ascontiguousarray(x[c * NIMG : (c + 1) * NIMG]),
            "w": w,
            "scal": sxw,
            "gamma2": g2,
            "beta2": b2,
        })

    res = run_bass_kernel_spmd(nc, in_maps, list(range(NCORES)), trace=_trace)
    out = np.concatenate([res.results[c]["out_val"] for c in range(NCORES)], axis=0)
    out = out.reshape(N, COUT, OH, OW)
    exp2 = np.float32(res.results[0]["out_exp"][0, 0])
    if _trace:
        kernel.last_results = res
    return out, exp2
